# revision 21
# baseline (speedup 1.0000x reference)
"""Trainium2 Bass kernel for the box-smoothed Charbonnier loss.

reference:  diff = conv7x7_box(sum_ch(x - y)) / 49 ;  loss = mean(sqrt(diff^2 + 1e-6))

Strategy (pure data parallel, 2 images per core on 8 cores):
  - Row-interleaved ("p-major") SBUF layout: partition p holds rows
    4p..4p+3, so DRAM runs are 8KB-contiguous. Loads are 1MB per-channel
    pieces, paired across the two HWDGE rings (x on SP, y on ACT) so the
    DVE difference/channel-sum chain streams behind the DMAs.
  - 7-wide box conv in each direction is a banded-matrix matmul on the PE
    in float32r (1 cycle/col vs 4 for fp32 at N=512). Band rides as the
    moving operand, image data as the stationary one, fusing conv+transpose.
    Strided column selection keeps both stages on the single p-major band:
        stage1[m, n] = sum_r s[r, 4m+cb] * band(r, n)    -> t partitions are w=4m+cb
        stage2[m, n] = sum_w t[w, 4m+hb] * band(w, n)    -> final rows h=4m+hb
  - Charbonnier on ACT: Square (PSUM->SBUF), Sqrt(x + eps) with accum_out
    collecting per-partition sums into acc[128, 8]; acc is DMA'd out and
    the host reduces it (with the cross-core sum) in float64.
"""

import numpy as np

import concourse.bass as bass
import concourse.bacc as bacc
import concourse.mybir as mybir
import concourse.tile as tile
from concourse.bass_interp import get_hw_module
from concourse.bass_utils import run_bass_kernel_spmd

N_CORES = 8
B_TOTAL = 16
B_PER_CORE = B_TOTAL // N_CORES
CH = 3
H = W = 512
P = 128
NCHUNK = H // P  # 4
EPS = 1e-6
F32 = mybir.dt.float32
F32R = mybir.dt.float32r
AF = mybir.ActivationFunctionType


def make_band() -> np.ndarray:
    """[128, 4, 512] p-major band: band[p, slot, n] = 1/7 if |4p+slot-n| <= 3."""
    band = np.zeros((P, NCHUNK, W), dtype=np.float32)
    p = np.arange(P)[:, None, None]
    slot = np.arange(NCHUNK)[None, :, None]
    n = np.arange(W)[None, None, :]
    band[np.abs(4 * p + slot - n) <= 3] = np.float32(1.0) / np.float32(7.0)
    return band


def build_program() -> tuple[bacc.Bacc, str, str, str, str]:
    nc = bacc.Bacc("TRN2", target_bir_lowering=False, debug=False, num_devices=N_CORES)

    x = nc.dram_tensor("x", [B_PER_CORE, CH, H, W], F32, kind="ExternalInput")
    y = nc.dram_tensor("y", [B_PER_CORE, CH, H, W], F32, kind="ExternalInput")
    band = nc.dram_tensor("band", [P, NCHUNK, W], F32R, kind="ExternalInput")
    out = nc.dram_tensor("out", [P, B_PER_CORE * NCHUNK], F32, kind="ExternalOutput")

    with tile.TileContext(nc) as tc:
        with (
            tc.tile_pool(name="const", bufs=1) as cpool,
            tc.tile_pool(name="xy", bufs=1) as xypool,
            tc.tile_pool(name="data", bufs=2) as dpool,
            tc.tile_pool(name="small", bufs=2) as spool,
            tc.tile_pool(name="psum", bufs=2, space="PSUM") as ppool,
        ):
            # band halves ride at the head of each HWDGE ring
            band_t = cpool.tile([P, NCHUNK, W], F32R)
            nc.sync.dma_start(band_t[:, 0:2, :], band.ap()[:, 0:2, :])
            nc.scalar.dma_start(band_t[:, 2:4, :], band.ap()[:, 2:4, :])

            epsb = cpool.tile([P, 1], F32)
            nc.gpsimd.memset(epsb[:], float(EPS))
            # pin the ACT table set (sqrt_and_others covers Copy/Square/Sqrt)
            # so no mid-kernel ACT_TABLE_LOAD lands on the critical path
            warm = cpool.tile([P, 1], F32)
            nc.scalar.activation(warm[:], epsb[:], AF.Sqrt)

            acc = cpool.tile([P, B_PER_CORE * NCHUNK], F32)

            # per-channel 1MB pieces: x on the SP ring, y on the ACT ring,
            # issued image-by-image so pieces pair up in time.
            xt, yt = [], []
            for b in range(B_PER_CORE):
                xb = xypool.tile([P, CH, NCHUNK, W], F32, tag=f"x{b}")
                yb = xypool.tile([P, CH, NCHUNK, W], F32, tag=f"y{b}")
                for ch in range(CH):
                    nc.sync.dma_start(
                        xb[:, ch, :, :],
                        x.ap()[b, ch].rearrange("(p c) w -> p c w", c=NCHUNK),
                    )
                    nc.scalar.dma_start(
                        yb[:, ch, :, :],
                        y.ap()[b, ch].rearrange("(p c) w -> p c w", c=NCHUNK),
                    )
                xt.append(xb)
                yt.append(yb)

            for b in range(B_PER_CORE):
                xb, yb = xt[b], yt[b]
                # s = (x0+x1+x2) - y0 - y1 - y2, ordered by piece arrival so
                # only the final subtract depends on the last-arriving piece
                st = dpool.tile([P, NCHUNK, W], F32, tag="st")
                s = dpool.tile([P, NCHUNK, W // 4, 4], F32R, tag="s")
                sv = s.rearrange("p c w4 f -> p c (w4 f)")
                nc.vector.tensor_add(st[:], xb[:, 0, :, :], xb[:, 1, :, :])
                nc.vector.tensor_add(st[:], st[:], xb[:, 2, :, :])
                nc.vector.tensor_sub(st[:], st[:], yb[:, 0, :, :])
                nc.vector.tensor_sub(st[:], st[:], yb[:, 1, :, :])
                nc.vector.tensor_sub(sv[:], st[:], yb[:, 2, :, :])

                # stage 1: vertical conv + transpose; column-select w = 4m+cb
                t = dpool.tile([P, NCHUNK, W // 4, 4], F32R, tag="t")
                for cb in range(NCHUNK):
                    ps1 = ppool.tile([P, W], F32, tag="ps1")
                    for c in range(NCHUNK):
                        nc.tensor.matmul(
                            ps1[:],
                            s[:, c, :, cb],
                            band_t[:, c, :],
                            start=(c == 0),
                            stop=(c == NCHUNK - 1),
                        )
                    nc.scalar.copy(
                        t[:, cb, :, :].rearrange("p w4 f -> p (w4 f)"), ps1[:]
                    )

                # stage 2: horizontal conv, rows back as h = 4m+hb
                for hb in range(NCHUNK):
                    ps2 = ppool.tile([P, W], F32, tag="ps2")
                    for cb in range(NCHUNK):
                        nc.tensor.matmul(
                            ps2[:],
                            t[:, cb, :, hb],
                            band_t[:, cb, :],
                            start=(cb == 0),
                            stop=(cb == NCHUNK - 1),
                        )
                    sq = spool.tile([P, W], F32, tag="sq")
                    nc.scalar.activation(sq[:], ps2[:], AF.Square)
                    u = spool.tile([P, W], F32, tag="u")
                    col = b * NCHUNK + hb
                    nc.scalar.activation(
                        u[:], sq[:], AF.Sqrt, bias=epsb[:],
                        accum_out=acc[:, col:col + 1],
                    )

            nc.sync.dma_start(out.ap()[:], acc[:])

    nc.compile()
    nc.m = get_hw_module(nc.m)
    return nc, x.name, y.name, band.name, out.name


_CACHE = {}


def _get_program():
    if "prog" not in _CACHE:
        _CACHE["prog"] = build_program()
    return _CACHE["prog"]


def run_sharded(x: np.ndarray, y: np.ndarray, trace: bool = False):
    """Run the SPMD kernel; returns (per-core sums list, BassKernelResults)."""
    nc, xname, yname, bandname, outname = _get_program()
    band = make_band()
    x = np.ascontiguousarray(np.asarray(x, dtype=np.float32))
    y = np.ascontiguousarray(np.asarray(y, dtype=np.float32))
    in_maps = []
    for k in range(N_CORES):
        sl = slice(k * B_PER_CORE, (k + 1) * B_PER_CORE)
        in_maps.append({
            xname: x[sl],
            yname: y[sl],
            bandname: band,
        })
    res = run_bass_kernel_spmd(
        nc, in_maps, core_ids=list(range(N_CORES)), trace=trace
    )
    sums = [float(res.results[k][outname].astype(np.float64).sum())
            for k in range(N_CORES)]
    return sums, res


def kernel(x: np.ndarray, y: np.ndarray) -> np.ndarray:
    sums, _ = run_sharded(x, y)
    total = float(np.sum(np.asarray(sums, dtype=np.float64)))
    return np.float32(total / (B_TOTAL * H * W))


# revision 23
# speedup vs baseline: 1.1697x; 1.1697x over previous
"""Trainium2 Bass kernel for the box-smoothed Charbonnier loss.

reference:  diff = conv7x7_box(sum_ch(x - y)) / 49 ;  loss = mean(sqrt(diff^2 + 1e-6))

Strategy (pure data parallel, 2 images per core on 8 cores):
  - Row-interleaved ("p-major") SBUF layout: partition p holds rows
    4p..4p+3, so DRAM runs are 8KB-contiguous. Loads are 1MB per-channel
    pieces, paired across the two HWDGE rings (x on SP, y on ACT) so the
    DVE difference/channel-sum chain streams behind the DMAs.
  - 7-wide box conv in each direction is a banded-matrix matmul on the PE
    in float32r (1 cycle/col vs 4 for fp32 at N=512). Band rides as the
    moving operand, image data as the stationary one, fusing conv+transpose.
    Strided column selection keeps both stages on the single p-major band:
        stage1[m, n] = sum_r s[r, 4m+cb] * band(r, n)    -> t partitions are w=4m+cb
        stage2[m, n] = sum_w t[w, 4m+hb] * band(w, n)    -> final rows h=4m+hb
  - Charbonnier on ACT: Square (PSUM->SBUF), Sqrt(x + eps) with accum_out
    collecting per-partition sums into acc[128, 8]; acc is DMA'd out and
    the host reduces it (with the cross-core sum) in float64.
"""

import numpy as np

import concourse.bass as bass
import concourse.bacc as bacc
import concourse.mybir as mybir
import concourse.tile as tile
from concourse.bass_interp import get_hw_module
from concourse.bass_utils import run_bass_kernel_spmd

N_CORES = 8
B_TOTAL = 16
B_PER_CORE = B_TOTAL // N_CORES
CH = 3
H = W = 512
P = 128
NCHUNK = H // P  # 4
EPS = 1e-6
F32 = mybir.dt.float32
F32R = mybir.dt.float32r
AF = mybir.ActivationFunctionType


def make_band() -> np.ndarray:
    """[128, 4, 512] p-major band: band[p, slot, n] = 1/7 if |4p+slot-n| <= 3."""
    band = np.zeros((P, NCHUNK, W), dtype=np.float32)
    p = np.arange(P)[:, None, None]
    slot = np.arange(NCHUNK)[None, :, None]
    n = np.arange(W)[None, None, :]
    band[np.abs(4 * p + slot - n) <= 3] = np.float32(1.0) / np.float32(7.0)
    return band


def build_program() -> tuple[bacc.Bacc, str, str, str, str]:
    nc = bacc.Bacc("TRN2", target_bir_lowering=False, debug=False, num_devices=N_CORES)

    x = nc.dram_tensor("x", [B_PER_CORE, CH, H, W], F32, kind="ExternalInput")
    y = nc.dram_tensor("y", [B_PER_CORE, CH, H, W], F32, kind="ExternalInput")
    band = nc.dram_tensor("band", [P, NCHUNK, W], F32R, kind="ExternalInput")
    out = nc.dram_tensor("out", [P, B_PER_CORE * NCHUNK], F32, kind="ExternalOutput")

    with tile.TileContext(nc) as tc:
        with (
            tc.tile_pool(name="const", bufs=1) as cpool,
            tc.tile_pool(name="xy", bufs=1) as xypool,
            tc.tile_pool(name="data", bufs=2) as dpool,
            tc.tile_pool(name="small", bufs=2) as spool,
            tc.tile_pool(name="psum", bufs=2, space="PSUM") as ppool,
        ):
            # band halves ride at the head of each HWDGE ring
            band_t = cpool.tile([P, NCHUNK, W], F32R)
            nc.sync.dma_start(band_t[:, 0:2, :], band.ap()[:, 0:2, :])
            nc.scalar.dma_start(band_t[:, 2:4, :], band.ap()[:, 2:4, :])

            epsb = cpool.tile([P, 1], F32)
            nc.gpsimd.memset(epsb[:], float(EPS))
            # pin the ACT table set (sqrt_and_others covers Copy/Square/Sqrt)
            # so no mid-kernel ACT_TABLE_LOAD lands on the critical path
            warm = cpool.tile([P, 1], F32)
            nc.scalar.activation(warm[:], epsb[:], AF.Sqrt)

            acc = cpool.tile([P, B_PER_CORE * NCHUNK], F32)

            # per-channel 1MB pieces: x on the SP ring, y on the ACT ring,
            # issued image-by-image so pieces pair up in time.
            xt, yt = [], []
            for b in range(B_PER_CORE):
                xb = xypool.tile([P, CH, NCHUNK, W], F32, tag=f"x{b}")
                yb = xypool.tile([P, CH, NCHUNK, W], F32, tag=f"y{b}")
                for ch in range(CH):
                    nc.sync.dma_start(
                        xb[:, ch, :, :],
                        x.ap()[b, ch].rearrange("(p c) w -> p c w", c=NCHUNK),
                    )
                    nc.scalar.dma_start(
                        yb[:, ch, :, :],
                        y.ap()[b, ch].rearrange("(p c) w -> p c w", c=NCHUNK),
                    )
                xt.append(xb)
                yt.append(yb)

            prev_dve = None

            def dve_ordered(inst):
                # pin the DVE queue to piece-arrival order: the scheduler's
                # cost model mis-predicts DMA completion and otherwise puts
                # data-starved ops ahead of ready ones (in-order engine).
                nonlocal prev_dve
                if prev_dve is not None:
                    tile.add_dep_helper(inst.ins, prev_dve, sync=False,
                                        reason="dve arrival order")
                prev_dve = inst.ins
                return inst

            for b in range(B_PER_CORE):
                xb, yb = xt[b], yt[b]
                # s = sum_ch (x - y); per-channel subs as piece pairs arrive,
                # partial add between, so only d2 + final add trail the last piece
                d = xypool.tile([P, CH, NCHUNK, W], F32, tag="d")
                e = dpool.tile([P, NCHUNK, W], F32, tag="e")
                s = dpool.tile([P, NCHUNK, W // 4, 4], F32R, tag="s")
                sv = s.rearrange("p c w4 f -> p c (w4 f)")
                dve_ordered(nc.vector.tensor_sub(
                    d[:, 0, :, :], xb[:, 0, :, :], yb[:, 0, :, :]))
                dve_ordered(nc.vector.tensor_sub(
                    d[:, 1, :, :], xb[:, 1, :, :], yb[:, 1, :, :]))
                dve_ordered(nc.vector.tensor_add(
                    e[:], d[:, 0, :, :], d[:, 1, :, :]))
                dve_ordered(nc.vector.tensor_sub(
                    d[:, 2, :, :], xb[:, 2, :, :], yb[:, 2, :, :]))
                dve_ordered(nc.vector.tensor_add(sv[:], e[:], d[:, 2, :, :]))

                # stage 1: vertical conv + transpose; column-select w = 4m+cb
                t = dpool.tile([P, NCHUNK, W // 4, 4], F32R, tag="t")
                for cb in range(NCHUNK):
                    ps1 = ppool.tile([P, W], F32, tag="ps1")
                    for c in range(NCHUNK):
                        nc.tensor.matmul(
                            ps1[:],
                            s[:, c, :, cb],
                            band_t[:, c, :],
                            start=(c == 0),
                            stop=(c == NCHUNK - 1),
                        )
                    nc.scalar.copy(
                        t[:, cb, :, :].rearrange("p w4 f -> p (w4 f)"), ps1[:]
                    )

                # stage 2: horizontal conv, rows back as h = 4m+hb
                for hb in range(NCHUNK):
                    ps2 = ppool.tile([P, W], F32, tag="ps2")
                    for cb in range(NCHUNK):
                        nc.tensor.matmul(
                            ps2[:],
                            t[:, cb, :, hb],
                            band_t[:, cb, :],
                            start=(cb == 0),
                            stop=(cb == NCHUNK - 1),
                        )
                    sq = spool.tile([P, W], F32, tag="sq")
                    nc.scalar.activation(sq[:], ps2[:], AF.Square)
                    u = spool.tile([P, W], F32, tag="u")
                    col = b * NCHUNK + hb
                    nc.scalar.activation(
                        u[:], sq[:], AF.Sqrt, bias=epsb[:],
                        accum_out=acc[:, col:col + 1],
                    )

            nc.sync.dma_start(out.ap()[:], acc[:])

    nc.compile()
    nc.m = get_hw_module(nc.m)
    return nc, x.name, y.name, band.name, out.name


_CACHE = {}


def _get_program():
    if "prog" not in _CACHE:
        _CACHE["prog"] = build_program()
    return _CACHE["prog"]


def run_sharded(x: np.ndarray, y: np.ndarray, trace: bool = False):
    """Run the SPMD kernel; returns (per-core sums list, BassKernelResults)."""
    nc, xname, yname, bandname, outname = _get_program()
    band = make_band()
    x = np.ascontiguousarray(np.asarray(x, dtype=np.float32))
    y = np.ascontiguousarray(np.asarray(y, dtype=np.float32))
    in_maps = []
    for k in range(N_CORES):
        sl = slice(k * B_PER_CORE, (k + 1) * B_PER_CORE)
        in_maps.append({
            xname: x[sl],
            yname: y[sl],
            bandname: band,
        })
    res = run_bass_kernel_spmd(
        nc, in_maps, core_ids=list(range(N_CORES)), trace=trace
    )
    sums = [float(res.results[k][outname].astype(np.float64).sum())
            for k in range(N_CORES)]
    return sums, res


def kernel(x: np.ndarray, y: np.ndarray) -> np.ndarray:
    sums, _ = run_sharded(x, y)
    total = float(np.sum(np.asarray(sums, dtype=np.float64)))
    return np.float32(total / (B_TOTAL * H * W))
